# revision 1
# baseline (speedup 1.0000x reference)
"""Fused multi-head cross-attention for Trainium2, SPMD over 8 NeuronCores.

Problem: out = softmax(q @ k^T / sqrt(D) + attn_bias + pad_mask) @ v
  q: (B=4, Sq=2048, H=16, D=128) f32, kv: (B, Sk=2048, 2, H, D) f32,
  attn_bias: (B, Sk) f32, key_padding_mask: (B, Sk) bool -> out (B, Sq, H, D) f32

Sharding: 64 (b, h) slices; core k owns batch k//2, heads (k%2)*8..+8.

Per-core kernel (per head slice):
  - Q, K are host-cast to bf16 and DMA-transposed into D-major layout
    (D on partitions).  S^T = K @ Q^T is computed chunk-by-chunk with the
    Sk-chunk on PSUM partitions and q on the free axis.  In this layout the
    per-key attn_bias is a per-partition vector, so it fuses into the ACT
    exponential (exp(scale * s + bias)) at zero cost.
  - V is loaded naturally (Sk on partitions) with a ones-column appended.
    out_ext^T-free accumulation: out_ext = P'^T_chunk.T @ [V | 1] accumulates
    over chunks in fp32 PSUM and yields BOTH the unnormalized output in
    natural (q, D) layout AND the softmax denominator in column D.
  - DVE computes 1/l and applies it as a per-partition scalar multiply.
"""

import sys

if "/opt/trn_rl_repo" not in sys.path:
    sys.path.insert(0, "/opt/trn_rl_repo")

import numpy as np
import ml_dtypes

B, SQ, SK, H, D = 4, 2048, 2048, 16, 128
NCORES = 8
NSL = H * B // NCORES  # 8 head-slices per core
CK = SK // 128  # 16 sk chunks
NQT = SQ // 128  # 16 q tiles of 128
QH = SQ // 1024  # 2 q halves (1024 wide) for the S^T psum tiles
SCALE = float(1.0 / np.sqrt(np.float32(D)))

_CACHE = {}


def _build_nc(nrep=1, pss_bufs=2, pso_bufs=2, lead=2, ot_bufs=4, grp=3):
    """nrep > 1 repeats the whole per-core computation (same inputs/outputs)
    back-to-back; used only for wall-clock timing (device work >> RPC cost)."""
    import concourse.bacc as bacc
    import concourse.tile as tile
    import concourse.mybir as mybir

    f32 = mybir.dt.float32
    bf16 = mybir.dt.bfloat16

    nc = bacc.Bacc("TRN2", target_bir_lowering=False, debug=False)
    qd = nc.dram_tensor("qb", [NSL, SQ, D], bf16, kind="ExternalInput").ap()
    kd = nc.dram_tensor("kb", [NSL, SK, D], bf16, kind="ExternalInput").ap()
    vd = nc.dram_tensor("vb", [NSL, SK, D], bf16, kind="ExternalInput").ap()
    # exp(attn_bias + mask) per key, laid out (sk%128, chunk)
    bd = nc.dram_tensor("ebT", [128, CK], f32, kind="ExternalInput").ap()
    od = nc.dram_tensor("out", [NSL, SQ, D], f32, kind="ExternalOutput").ap()

    with tile.TileContext(nc) as tc:
        with (
            tc.tile_pool(name="qt", bufs=3) as qt_pool,
            tc.tile_pool(name="kt", bufs=3) as kt_pool,
            tc.tile_pool(name="vp", bufs=3) as vp_pool,
            tc.tile_pool(name="pp", bufs=2) as pp_pool,
            tc.tile_pool(name="bias", bufs=1) as bias_pool,
            tc.tile_pool(name="ot", bufs=ot_bufs) as ot_pool,
            tc.tile_pool(name="rc", bufs=ot_bufs) as rc_pool,
            tc.tile_pool(name="psS", bufs=pss_bufs, space="PSUM") as psS_pool,  # 3 banks each
            tc.tile_pool(name="psO", bufs=pso_bufs, space="PSUM") as psO_pool,
        ):
            bias_sb = bias_pool.tile([128, CK], f32)
            nc.sync.dma_start(bias_sb[:], bd[:])

            slice_tiles = {}

            def load_slice(s):
                # Q^T, K^T in D-major layout via hardware transpose DMA.
                qt_t = qt_pool.tile([128, SQ], bf16)
                nc.sync.dma_start(qt_t[:], qd[s], transpose=True)
                kt_t = kt_pool.tile([128, SK], bf16)
                nc.sync.dma_start(kt_t[:], kd[s], transpose=True)
                # V natural (sk%128 on partitions), ones column appended;
                # every column (incl. the ones) scaled by exp(bias + mask) so
                # the bias drops out of the exponential: the PV matmul then
                # computes sum_k exp(s)*e^b*V and sum_k exp(s)*e^b directly.
                vp_t = vp_pool.tile([128, CK, D + 1], bf16)
                nc.sync.dma_start(
                    vp_t[:, :, 0:D], vd[s].rearrange("(c p) d -> p c d", p=128)
                )
                nc.vector.memset(vp_t[:, :, D : D + 1], 1.0)
                for c in range(CK):
                    nc.vector.tensor_scalar_mul(
                        vp_t[:, c, :], vp_t[:, c, :], bias_sb[:, c : c + 1]
                    )
                pp_t = pp_pool.tile([128, CK, SQ], bf16)
                slice_tiles[s] = (qt_t, kt_t, vp_t, pp_t)

            # S^T 512-wide blocks in flat order b = c*4 + qcol; both the pp
            # free offset (c*SQ + qcol*512) and the block index advance by
            # 512 per block, so any run of consecutive blocks is contiguous
            # in pp and can be exp'd by a single ACT op.
            GRP = grp  # psum banks (512 f32 each) per exp op

            def s_group(s, b0, nblk):
                qt_t, kt_t, _, pp_t = slice_tiles[s]
                ps = psS_pool.tile([128, GRP * 512], f32)
                for j in range(nblk):
                    b = b0 + j
                    c, qcol = divmod(b, SQ // 512)
                    nc.tensor.matmul(
                        ps[:, j * 512 : (j + 1) * 512],
                        lhsT=kt_t[:, c * 128 : (c + 1) * 128],
                        rhs=qt_t[:, qcol * 512 : (qcol + 1) * 512],
                        start=True,
                        stop=True,
                    )
                pp_flat = pp_t.rearrange("p c q -> p (c q)")
                nc.scalar.activation(
                    pp_flat[:, b0 * 512 : (b0 + nblk) * 512],
                    ps[:, 0 : nblk * 512],
                    mybir.ActivationFunctionType.Exp,
                    scale=SCALE,
                )

            def pv_tile(s, t):
                # out_ext = P'^T.T @ [V|1] accumulated over chunks; normalize.
                _, _, vp_t, pp_t = slice_tiles[s]
                po = psO_pool.tile([128, D + 1], f32)
                for c in range(CK):
                    nc.tensor.matmul(
                        po[:],
                        lhsT=pp_t[:, c, t * 128 : (t + 1) * 128],
                        rhs=vp_t[:, c, :],
                        start=(c == 0),
                        stop=(c == CK - 1),
                    )
                rc = rc_pool.tile([128, 1], f32)
                nc.vector.reciprocal(rc[:], po[:, D : D + 1])
                ot = ot_pool.tile([128, D], f32)
                nc.vector.tensor_scalar_mul(ot[:], po[:, 0:D], rc[:])
                nc.sync.dma_start(od[s, t * 128 : (t + 1) * 128, :], ot[:])

            # Software-pipelined emission at slice granularity: the ACT-bound
            # S groups of slice v interleave with the PE-only PV tiles of
            # slice v-1 so both engines stay fed.
            NBLK = CK * (SQ // 512)  # 64 blocks per slice
            groups = []
            b0 = 0
            while b0 < NBLK:
                n = min(GRP, NBLK - b0)
                groups.append((b0, n))
                b0 += n

            NV = NSL * nrep  # total virtual slices

            def emit_step(v):
                s = v % NSL
                gs = list(groups) if v < NV else []
                pvs = [((v - 1) % NSL, t) for t in range(NQT)] if v > 0 else []
                if v + 1 < NV:
                    load_slice((v + 1) % NSL)
                gi, pi = 0, 0
                acc = 0.0
                ratio = len(pvs) / max(1, len(gs)) if gs else 0.0
                for _ in range(min(lead, len(gs))):
                    s_group(s, *gs[gi])
                    gi += 1
                while gi < len(gs) or pi < len(pvs):
                    if gi < len(gs):
                        s_group(s, *gs[gi])
                        gi += 1
                        acc += ratio
                        while acc >= 1.0 and pi < len(pvs):
                            pv_tile(*pvs[pi])
                            pi += 1
                            acc -= 1.0
                    else:
                        pv_tile(*pvs[pi])
                        pi += 1

            load_slice(0)
            for v in range(NV + 1):
                emit_step(v)

    nc.compile()
    return nc


def _get_nc():
    if "nc" not in _CACHE:
        _CACHE["nc"] = _build_nc()
    return _CACHE["nc"]


def _make_in_maps(q, kv, attn_bias, key_padding_mask):
    q = np.asarray(q)
    kv = np.asarray(kv)
    attn_bias = np.asarray(attn_bias, dtype=np.float32)
    key_padding_mask = np.asarray(key_padding_mask)

    biasp = attn_bias + np.where(key_padding_mask, 0.0, -1e30).astype(np.float32)
    ebias = np.exp(biasp)  # masked keys -> exactly 0
    bf16 = ml_dtypes.bfloat16

    in_maps = []
    for core in range(NCORES):
        b = core // (NCORES // B)
        h0 = (core % (NCORES // B)) * NSL
        qb = np.ascontiguousarray(
            q[b, :, h0 : h0 + NSL, :].transpose(1, 0, 2)
        ).astype(bf16)
        kb = np.ascontiguousarray(
            kv[b, :, 0, h0 : h0 + NSL, :].transpose(1, 0, 2)
        ).astype(bf16)
        vb = np.ascontiguousarray(
            kv[b, :, 1, h0 : h0 + NSL, :].transpose(1, 0, 2)
        ).astype(bf16)
        ebT = np.ascontiguousarray(ebias[b].reshape(CK, 128).T.astype(np.float32))
        in_maps.append({"qb": qb, "kb": kb, "vb": vb, "ebT": ebT})
    return in_maps


def _gather(results):
    out = np.empty((B, SQ, H, D), dtype=np.float32)
    for core in range(NCORES):
        b = core // (NCORES // B)
        h0 = (core % (NCORES // B)) * NSL
        out[b, :, h0 : h0 + NSL, :] = results[core]["out"].transpose(1, 0, 2)
    return out


def kernel(q, kv, attn_bias, key_padding_mask):
    from concourse.bass_utils import run_bass_kernel_spmd

    nc = _get_nc()
    in_maps = _make_in_maps(q, kv, attn_bias, key_padding_mask)
    res = run_bass_kernel_spmd(nc, in_maps, list(range(NCORES)))
    return _gather(res.results)



# revision 13
# speedup vs baseline: 1.1768x; 1.1768x over previous
"""Fused multi-head cross-attention for Trainium2, SPMD over 8 NeuronCores.

Problem: out = softmax(q @ k^T / sqrt(D) + attn_bias + pad_mask) @ v
  q: (B=4, Sq=2048, H=16, D=128) f32, kv: (B, Sk=2048, 2, H, D) f32,
  attn_bias: (B, Sk) f32, key_padding_mask: (B, Sk) bool -> out (B, Sq, H, D) f32

Sharding: 64 (b, h) slices; core k owns batch k//2, heads (k%2)*8..+8.

Per-core kernel (per head slice, pipelined at HALF-slice granularity):
  - Q, K host-cast to bf16, DMA-transposed into D-major layout (D on
    partitions).  S^T = K @ Q^T is computed in 512-wide blocks with the
    Sk-chunk on PSUM partitions; block order within a 1024-wide q-half is
    (qcol, c) so K chunks are needed progressively.
  - exp: most 3-block groups go through the ACT engine (exp(SCALE*s) with
    bf16 output); a tunable subset runs on the otherwise-idle DVE as a
    single tensor_scalar op computing round(s*(SCALE*2^7*log2 e) +
    (127*2^7 - C)) into an int16 view of the bf16 P tile — Schraudolph's
    exponent-bitfield exp evaluated directly in bf16 bit-space (~1.8% rms,
    which softmax normalization largely cancels; overall output err stays
    well under the 2e-2 gate).
  - attn_bias+mask fold into V on the HOST: vb is staged pre-chunked as
    [128, CK, D+1] bf16 with every column (plus an appended ones column)
    pre-scaled by exp(bias).  The PV matmul then yields both the
    unnormalized output and the softmax denominator, and the V load is a
    single contiguous DMA.
  - PV: out_ext = P'^T_chunk.T @ [V|1] accumulated over chunks in PSUM,
    per 128-row q tile; DVE computes 1/l and applies it.
"""

import sys

if "/opt/trn_rl_repo" not in sys.path:
    sys.path.insert(0, "/opt/trn_rl_repo")

import numpy as np
import ml_dtypes

B, SQ, SK, H, D = 4, 2048, 2048, 16, 128
NCORES = 8
NSL = H * B // NCORES  # 8 head-slices per core
CK = SK // 128  # 16 sk chunks
NQT = SQ // 128  # 16 q tiles of 128
SQH = 1024  # q-half width
NH = 2  # halves per slice
SCALE = float(1.0 / np.sqrt(np.float32(D)))

# Schraudolph exp in bf16 bit-space: i16 = round(x*EXP_A + EXP_B) viewed
# as bf16 approximates exp(SCALE*x).  C tuned for min rms error.
EXP_C = 7.0
EXP_A = float(SCALE * (1 << 7) / np.log(2.0))
EXP_B = float(127.0 * (1 << 7) - EXP_C)

_CACHE = {}


def _build_nc(nrep=1, grp=3, dve_groups=(4, 9), lead=2):
    """nrep > 1 repeats the whole per-core computation (same inputs/outputs)
    back-to-back; used only for wall-clock timing (device work >> RPC cost)."""
    import concourse.bacc as bacc
    import concourse.tile as tile
    import concourse.mybir as mybir

    f32 = mybir.dt.float32
    bf16 = mybir.dt.bfloat16
    i16 = mybir.dt.int16

    nc = bacc.Bacc("TRN2", target_bir_lowering=False, debug=False)
    qd = nc.dram_tensor("qb", [NSL, SQ, D], bf16, kind="ExternalInput").ap()
    kd = nc.dram_tensor("kb", [NSL, SK, D], bf16, kind="ExternalInput").ap()
    # V pre-chunked and pre-scaled by exp(bias+mask), ones column appended
    vd = nc.dram_tensor("vb", [NSL, 128, CK, D + 1], bf16, kind="ExternalInput").ap()
    od = nc.dram_tensor("out", [NSL, SQ, D], f32, kind="ExternalOutput").ap()

    NBH = 2 * CK  # 32 S blocks per half, order (qcol, c)

    with tile.TileContext(nc) as tc:
        with (
            tc.tile_pool(name="qt", bufs=3) as qt_pool,
            tc.tile_pool(name="kt", bufs=3) as kt_pool,
            tc.tile_pool(name="vp", bufs=3) as vp_pool,
            tc.tile_pool(name="pp", bufs=2) as pp_pool,
            tc.tile_pool(name="ot", bufs=6) as ot_pool,
            tc.tile_pool(name="rc", bufs=4) as rc_pool,
            tc.tile_pool(name="psS", bufs=2, space="PSUM") as psS_pool,
            tc.tile_pool(name="psO", bufs=2, space="PSUM") as psO_pool,
        ):
            from collections import deque

            slice_tiles = {}
            out_q = deque()

            def load_slice(s):
                # DMA loads only — issued a full half-slice before the data
                # is needed.  DMAs ordered by first use: the first S group
                # needs qt cols [0,512) and kt piece 0 (the tile dep tracker
                # gates readers at subtile level).
                qt_t = qt_pool.tile([128, SQ], bf16)
                kt_t = kt_pool.tile([128, SK], bf16)
                nc.sync.dma_start(
                    qt_t[:, 0:512], qd[s, 0:512, :], transpose=True
                )
                for p in range(4):
                    nc.sync.dma_start(
                        kt_t[:, p * 512 : (p + 1) * 512],
                        kd[s, p * 512 : (p + 1) * 512, :],
                        transpose=True,
                    )
                nc.sync.dma_start(
                    qt_t[:, 512:SQH], qd[s, 512:SQH, :], transpose=True
                )
                nc.sync.dma_start(
                    qt_t[:, SQH:SQ], qd[s, SQH:SQ, :], transpose=True
                )
                vp_t = vp_pool.tile([128, CK, D + 1], bf16)
                nc.sync.dma_start(vp_t[:], vd[s])
                pp_a = pp_pool.tile([128, 2, CK, 512], bf16, name="pp_a")
                pp_b = pp_pool.tile([128, 2, CK, 512], bf16, name="pp_b")
                slice_tiles[s] = (qt_t, kt_t, vp_t, (pp_a, pp_b))

            def s_group(s, qh, b0, nblk, engine):
                # S^T blocks b = qcol*CK + c of half qh; pp free offset is
                # qcol*(CK*512) + c*512 = b*512, so any run of consecutive
                # blocks is contiguous and one ACT/DVE op covers the group.
                qt_t, kt_t, _, pps = slice_tiles[s]
                pp_t = pps[qh]
                ps = psS_pool.tile([128, grp * 512], f32)
                for j in range(nblk):
                    b = b0 + j
                    qcol, c = divmod(b, CK)
                    q0 = qh * SQH + qcol * 512
                    nc.tensor.matmul(
                        ps[:, j * 512 : (j + 1) * 512],
                        lhsT=kt_t[:, c * 128 : (c + 1) * 128],
                        rhs=qt_t[:, q0 : q0 + 512],
                        start=True,
                        stop=True,
                    )
                pp_flat = pp_t.rearrange("p a c q -> p (a c q)")
                if engine == "act":
                    nc.scalar.activation(
                        pp_flat[:, b0 * 512 : (b0 + nblk) * 512],
                        ps[:, 0 : nblk * 512],
                        mybir.ActivationFunctionType.Exp,
                        scale=SCALE,
                    )
                else:
                    pp_bits = pp_flat.bitcast(i16)
                    nc.vector.tensor_scalar(
                        pp_bits[:, b0 * 512 : (b0 + nblk) * 512],
                        ps[:, 0 : nblk * 512],
                        EXP_A,
                        EXP_B,
                        op0=mybir.AluOpType.mult,
                        op1=mybir.AluOpType.add,
                    )

            def pv_tile(s, qh, t):
                # out_ext = P'^T.T @ [V|1] accumulated over chunks; normalize.
                # t in [0, 8): q rows [qh*SQH + t*128, +128).
                _, _, vp_t, pps = slice_tiles[s]
                pp_t = pps[qh]
                po = psO_pool.tile([128, D + 1], f32)
                toff = t * 128
                qcol, col = divmod(toff, 512)
                for c in range(CK):
                    nc.tensor.matmul(
                        po[:],
                        lhsT=pp_t[:, qcol, c, col : col + 128],
                        rhs=vp_t[:, c, :],
                        start=(c == 0),
                        stop=(c == CK - 1),
                    )
                rc = rc_pool.tile([128, 1], f32)
                nc.vector.reciprocal(rc[:], po[:, D : D + 1])
                ot = ot_pool.tile([128, D], f32)
                nc.vector.tensor_scalar_mul(ot[:], po[:, 0:D], rc[:])
                # Defer the out DMA by a couple of PV tiles: by issue time
                # the DVE mul has completed, so the SP sequencer never
                # blocks on it (head-of-line for the next slice's loads).
                out_q.append(
                    lambda s=s, qh=qh, toff=toff, ot=ot: nc.sync.dma_start(
                        od[s, qh * SQH + toff : qh * SQH + toff + 128, :], ot[:]
                    )
                )
                while len(out_q) > 2:
                    out_q.popleft()()

            # Per-half S groups: 32 blocks in runs of grp.
            groups = []
            b0 = 0
            gi = 0
            while b0 < NBH:
                n = min(grp, NBH - b0)
                eng = "dve" if gi in dve_groups else "act"
                groups.append((b0, n, eng))
                b0 += n
                gi += 1

            NV = NSL * NH * nrep  # total virtual halves

            def emit_step(v):
                s, qh = (v // NH) % NSL, v % NH
                ps_, pqh = ((v - 1) // NH) % NSL, (v - 1) % NH
                gs = list(groups) if v < NV else []
                pvs = [(ps_, pqh, t) for t in range(8)] if v > 0 else []
                if qh == 0 and v + NH < NV:
                    # issue next slice's DMAs a full half ahead of use
                    load_slice((v // NH + 1) % NSL)
                gi_, pi = 0, 0
                acc = 0.0
                gratio = len(pvs) / max(1, len(gs)) if gs else 0.0
                for _ in range(min(lead, len(gs))):
                    s_group(s, qh, *gs[gi_])
                    gi_ += 1
                while gi_ < len(gs) or pi < len(pvs):
                    if gi_ < len(gs):
                        s_group(s, qh, *gs[gi_])
                        gi_ += 1
                        acc += gratio
                        while acc >= 1.0 and pi < len(pvs):
                            pv_tile(*pvs[pi])
                            pi += 1
                            acc -= 1.0
                    else:
                        pv_tile(*pvs[pi])
                        pi += 1

            load_slice(0)
            for v in range(NV + 1):
                emit_step(v)
            while out_q:
                out_q.popleft()()

    nc.compile()
    return nc


def _get_nc():
    if "nc" not in _CACHE:
        _CACHE["nc"] = _build_nc()
    return _CACHE["nc"]


def _make_in_maps(q, kv, attn_bias, key_padding_mask):
    q = np.asarray(q)
    kv = np.asarray(kv)
    attn_bias = np.asarray(attn_bias, dtype=np.float32)
    key_padding_mask = np.asarray(key_padding_mask)

    biasp = attn_bias + np.where(key_padding_mask, 0.0, -1e30).astype(np.float32)
    ebias = np.exp(biasp)  # masked keys -> exactly 0
    bf16 = ml_dtypes.bfloat16

    in_maps = []
    for core in range(NCORES):
        b = core // (NCORES // B)
        h0 = (core % (NCORES // B)) * NSL
        qb = np.ascontiguousarray(
            q[b, :, h0 : h0 + NSL, :].transpose(1, 0, 2)
        ).astype(bf16)
        kb = np.ascontiguousarray(
            kv[b, :, 0, h0 : h0 + NSL, :].transpose(1, 0, 2)
        ).astype(bf16)
        # V*e^b with the e^b ones-column appended, laid out (h, sk%128, c, d)
        v = kv[b, :, 1, h0 : h0 + NSL, :]  # (SK, NSL, D)
        vext = np.empty((SK, NSL, D + 1), np.float32)
        vext[:, :, 0:D] = v * ebias[b][:, None, None]
        vext[:, :, D] = ebias[b][:, None]
        # (SK, NSL, D+1) -> (NSL, 128, CK, D+1) with sk = c*128 + p
        vb = np.ascontiguousarray(
            vext.reshape(CK, 128, NSL, D + 1).transpose(2, 1, 0, 3)
        ).astype(bf16)
        in_maps.append({"qb": qb, "kb": kb, "vb": vb})
    return in_maps


def _gather(results):
    out = np.empty((B, SQ, H, D), dtype=np.float32)
    for core in range(NCORES):
        b = core // (NCORES // B)
        h0 = (core % (NCORES // B)) * NSL
        out[b, :, h0 : h0 + NSL, :] = results[core]["out"].transpose(1, 0, 2)
    return out


def kernel(q, kv, attn_bias, key_padding_mask):
    from concourse.bass_utils import run_bass_kernel_spmd

    nc = _get_nc()
    in_maps = _make_in_maps(q, kv, attn_bias, key_padding_mask)
    res = run_bass_kernel_spmd(nc, in_maps, list(range(NCORES)))
    return _gather(res.results)


# revision 25
# speedup vs baseline: 1.2866x; 1.0933x over previous
"""Fused multi-head cross-attention for Trainium2, SPMD over 8 NeuronCores.

Problem: out = softmax(q @ k^T / sqrt(D) + attn_bias + pad_mask) @ v
  q: (B=4, Sq=2048, H=16, D=128) f32, kv: (B, Sk=2048, 2, H, D) f32,
  attn_bias: (B, Sk) f32, key_padding_mask: (B, Sk) bool -> out (B, Sq, H, D) f32

Sharding: 64 (b, h) slices; core k owns batch k//2, heads (k%2)*8..+8.

Per-core kernel (per head slice, pipelined at HALF-slice granularity):
  - Q, K host-cast to bf16, DMA-transposed into D-major layout (D on
    partitions).  S^T = K @ Q^T is computed in 512-wide blocks with the
    Sk-chunk on PSUM partitions; block order within a 1024-wide q-half is
    (qcol, c) so K chunks are needed progressively.
  - exp: most 3-block groups go through the ACT engine (exp(SCALE*s) with
    bf16 output); a tunable subset runs on the otherwise-idle DVE as a
    single tensor_scalar op computing round(s*(SCALE*2^7*log2 e) +
    (127*2^7 - C)) into an int16 view of the bf16 P tile — Schraudolph's
    exponent-bitfield exp evaluated directly in bf16 bit-space (~1.8% rms,
    which softmax normalization largely cancels; overall output err stays
    well under the 2e-2 gate).
  - attn_bias+mask fold into V on the HOST: vb is staged pre-chunked as
    [128, CK, D+1] bf16 with every column (plus an appended ones column)
    pre-scaled by exp(bias).  The PV matmul then yields both the
    unnormalized output and the softmax denominator, and the V load is a
    single contiguous DMA.
  - PV: out_ext = P'^T_chunk.T @ [V|1] accumulated over chunks in PSUM,
    per 128-row q tile; DVE computes 1/l and applies it.
"""

import sys

if "/opt/trn_rl_repo" not in sys.path:
    sys.path.insert(0, "/opt/trn_rl_repo")

import numpy as np
import ml_dtypes

B, SQ, SK, H, D = 4, 2048, 2048, 16, 128
NCORES = 8
NSL = H * B // NCORES  # 8 head-slices per core
CK = SK // 128  # 16 sk chunks
NQT = SQ // 128  # 16 q tiles of 128
SQH = 1024  # q-half width
NH = 2  # halves per slice
SCALE = float(1.0 / np.sqrt(np.float32(D)))

# Schraudolph exp in bf16 bit-space: i16 = round(x*EXP_A + EXP_B) viewed
# as bf16 approximates exp(SCALE*x).  C tuned for min rms error.
EXP_C = 7.0
EXP_A = float(SCALE * (1 << 7) / np.log(2.0))
EXP_B = float(127.0 * (1 << 7) - EXP_C)

_CACHE = {}


def _build_nc(nrep=1, grp=2, dve_groups=(3, 7, 11, 15), lead=2, pss_bufs=3,
              first_dve=(1, 3, 5, 7, 9, 11, 13), last_dve=(1, 3, 5)):
    """nrep > 1 repeats the whole per-core computation (same inputs/outputs)
    back-to-back; used only for wall-clock timing (device work >> RPC cost)."""
    import concourse.bacc as bacc
    import concourse.tile as tile
    import concourse.mybir as mybir

    f32 = mybir.dt.float32
    bf16 = mybir.dt.bfloat16
    i16 = mybir.dt.int16

    nc = bacc.Bacc("TRN2", target_bir_lowering=False, debug=False)
    qd = nc.dram_tensor("qb", [NSL, SQ, D], bf16, kind="ExternalInput").ap()
    kd = nc.dram_tensor("kb", [NSL, SK, D], bf16, kind="ExternalInput").ap()
    # V pre-chunked and pre-scaled by exp(bias+mask), ones column appended
    vd = nc.dram_tensor("vb", [NSL, 128, CK, D + 1], bf16, kind="ExternalInput").ap()
    od = nc.dram_tensor("out", [NSL, SQ, D], f32, kind="ExternalOutput").ap()

    NBH = 2 * CK  # 32 S blocks per half, order (qcol, c)

    with tile.TileContext(nc) as tc:
        with (
            tc.tile_pool(name="qt", bufs=3) as qt_pool,
            tc.tile_pool(name="kt", bufs=3) as kt_pool,
            tc.tile_pool(name="vp", bufs=3) as vp_pool,
            tc.tile_pool(name="pp", bufs=2) as pp_pool,
            tc.tile_pool(name="ot", bufs=12) as ot_pool,
            tc.tile_pool(name="rc", bufs=8) as rc_pool,
            tc.tile_pool(name="psS", bufs=pss_bufs, space="PSUM") as psS_pool,
            tc.tile_pool(name="psO", bufs=2, space="PSUM") as psO_pool,
        ):
            from collections import deque

            slice_tiles = {}
            out_q = deque()

            def load_slice(s):
                # DMA loads only — issued a full half-slice before the data
                # is needed.  DMAs ordered by first use: the first S group
                # needs qt cols [0,512) and kt piece 0 (the tile dep tracker
                # gates readers at subtile level).
                qt_t = qt_pool.tile([128, SQ], bf16)
                kt_t = kt_pool.tile([128, SK], bf16)
                nc.sync.dma_start(
                    qt_t[:, 0:512], qd[s, 0:512, :], transpose=True
                )
                for p in range(4):
                    nc.sync.dma_start(
                        kt_t[:, p * 512 : (p + 1) * 512],
                        kd[s, p * 512 : (p + 1) * 512, :],
                        transpose=True,
                    )
                nc.sync.dma_start(
                    qt_t[:, 512:SQH], qd[s, 512:SQH, :], transpose=True
                )
                nc.sync.dma_start(
                    qt_t[:, SQH:SQ], qd[s, SQH:SQ, :], transpose=True
                )
                vp_t = vp_pool.tile([128, CK, D + 1], bf16)
                nc.sync.dma_start(vp_t[:], vd[s])
                pp_a = pp_pool.tile([128, 2, CK, 512], bf16, name="pp_a")
                pp_b = pp_pool.tile([128, 2, CK, 512], bf16, name="pp_b")
                slice_tiles[s] = (qt_t, kt_t, vp_t, (pp_a, pp_b))

            def s_group(s, qh, b0, nblk, engine):
                # S^T blocks b = qcol*CK + c of half qh; pp free offset is
                # qcol*(CK*512) + c*512 = b*512, so any run of consecutive
                # blocks is contiguous and one ACT/DVE op covers the group.
                qt_t, kt_t, _, pps = slice_tiles[s]
                pp_t = pps[qh]
                ps = psS_pool.tile([128, grp * 512], f32)
                for j in range(nblk):
                    b = b0 + j
                    qcol, c = divmod(b, CK)
                    q0 = qh * SQH + qcol * 512
                    nc.tensor.matmul(
                        ps[:, j * 512 : (j + 1) * 512],
                        lhsT=kt_t[:, c * 128 : (c + 1) * 128],
                        rhs=qt_t[:, q0 : q0 + 512],
                        start=True,
                        stop=True,
                    )
                pp_flat = pp_t.rearrange("p a c q -> p (a c q)")
                if engine == "act":
                    nc.scalar.activation(
                        pp_flat[:, b0 * 512 : (b0 + nblk) * 512],
                        ps[:, 0 : nblk * 512],
                        mybir.ActivationFunctionType.Exp,
                        scale=SCALE,
                    )
                else:
                    pp_bits = pp_flat.bitcast(i16)
                    nc.vector.tensor_scalar(
                        pp_bits[:, b0 * 512 : (b0 + nblk) * 512],
                        ps[:, 0 : nblk * 512],
                        EXP_A,
                        EXP_B,
                        op0=mybir.AluOpType.mult,
                        op1=mybir.AluOpType.add,
                    )

            def pv_tile(s, qh, t, defer=True):
                # out_ext = P'^T.T @ [V|1] accumulated over chunks; normalize.
                # t in [0, 8): q rows [qh*SQH + t*128, +128).
                _, _, vp_t, pps = slice_tiles[s]
                pp_t = pps[qh]
                po = psO_pool.tile([128, D + 1], f32)
                toff = t * 128
                qcol, col = divmod(toff, 512)
                for c in range(CK):
                    nc.tensor.matmul(
                        po[:],
                        lhsT=pp_t[:, qcol, c, col : col + 128],
                        rhs=vp_t[:, c, :],
                        start=(c == 0),
                        stop=(c == CK - 1),
                    )
                rc = rc_pool.tile([128, 1], f32)
                nc.vector.reciprocal(rc[:], po[:, D : D + 1])
                ot = ot_pool.tile([128, D], f32)
                nc.vector.tensor_scalar_mul(ot[:], po[:, 0:D], rc[:])
                # Defer the out DMA by a couple of PV tiles: by issue time
                # the DVE mul has completed, so the SP sequencer never
                # blocks on it (head-of-line for the next slice's loads).
                out_q.append(
                    lambda s=s, qh=qh, toff=toff, ot=ot: nc.sync.dma_start(
                        od[s, qh * SQH + toff : qh * SQH + toff + 128, :], ot[:]
                    )
                )
                while len(out_q) > (2 if defer else 0):
                    out_q.popleft()()

            # Per-half S groups: 32 blocks in runs of grp.
            def make_groups(dve_set):
                out, b0, gi = [], 0, 0
                while b0 < NBH:
                    n = min(grp, NBH - b0)
                    out.append((b0, n, "dve" if gi in dve_set else "act"))
                    b0 += n
                    gi += 1
                return out

            groups = make_groups(dve_groups)
            # First half: the pipeline is empty, so exp throughput is the
            # critical path — split the groups nearly evenly across ACT+DVE.
            groups_first = make_groups(first_dve)
            # Last half: ACT is idle at the end (no next half), so keep the
            # final groups on ACT and let the DVE drain the norms fast.
            groups_last = make_groups(last_dve)

            NV = NSL * NH * nrep  # total virtual halves

            def emit_step(v):
                s, qh = (v // NH) % NSL, v % NH
                ps_, pqh = ((v - 1) // NH) % NSL, (v - 1) % NH
                if v == 0:
                    gs = list(groups_first)
                elif v == NV - 1:
                    gs = list(groups_last)
                else:
                    gs = list(groups)
                # (s, qh, t, min_gi): a PV tile may only be EMITTED once
                # min_gi groups have been emitted — program order IS the
                # dependency order for the tile tracker, so emitting a read
                # before its writer exists silently reads stale data.
                pvs = [(ps_, pqh, t, 0) for t in range(8)] if v > 0 else []
                if v == NV - 1:
                    # Fold the final half's own PV tiles into this step to
                    # keep the PE busy while the last exp groups drain.
                    ngq0 = (CK + grp - 1) // grp  # groups covering qcol 0
                    pvs += [
                        (s, qh, t, ngq0 if t < 4 else len(gs)) for t in range(8)
                    ]
                if qh == 0 and v + NH < NV:
                    # issue next slice's DMAs a full half ahead of use
                    load_slice((v // NH + 1) % NSL)
                defer = v < NV - 1
                gi_, pi = 0, 0
                acc = 0.0
                gratio = len(pvs) / max(1, len(gs)) if gs else 0.0
                for _ in range(min(lead, len(gs))):
                    s_group(s, qh, *gs[gi_])
                    gi_ += 1
                while gi_ < len(gs) or pi < len(pvs):
                    if gi_ < len(gs):
                        s_group(s, qh, *gs[gi_])
                        gi_ += 1
                        acc += gratio
                        while (
                            acc >= 1.0
                            and pi < len(pvs)
                            and pvs[pi][3] <= gi_
                        ):
                            pv_tile(*pvs[pi][:3], defer=defer)
                            pi += 1
                            acc -= 1.0
                    else:
                        pv_tile(*pvs[pi][:3], defer=defer)
                        pi += 1

            load_slice(0)
            for v in range(NV):
                emit_step(v)
            while out_q:
                out_q.popleft()()

    nc.compile()
    return nc


def _get_nc():
    if "nc" not in _CACHE:
        _CACHE["nc"] = _build_nc()
    return _CACHE["nc"]


def _make_in_maps(q, kv, attn_bias, key_padding_mask):
    q = np.asarray(q)
    kv = np.asarray(kv)
    attn_bias = np.asarray(attn_bias, dtype=np.float32)
    key_padding_mask = np.asarray(key_padding_mask)

    biasp = attn_bias + np.where(key_padding_mask, 0.0, -1e30).astype(np.float32)
    ebias = np.exp(biasp)  # masked keys -> exactly 0
    bf16 = ml_dtypes.bfloat16

    in_maps = []
    for core in range(NCORES):
        b = core // (NCORES // B)
        h0 = (core % (NCORES // B)) * NSL
        qb = np.ascontiguousarray(
            q[b, :, h0 : h0 + NSL, :].transpose(1, 0, 2)
        ).astype(bf16)
        kb = np.ascontiguousarray(
            kv[b, :, 0, h0 : h0 + NSL, :].transpose(1, 0, 2)
        ).astype(bf16)
        # V*e^b with the e^b ones-column appended, laid out (h, sk%128, c, d)
        v = kv[b, :, 1, h0 : h0 + NSL, :]  # (SK, NSL, D)
        vext = np.empty((SK, NSL, D + 1), np.float32)
        vext[:, :, 0:D] = v * ebias[b][:, None, None]
        vext[:, :, D] = ebias[b][:, None]
        # (SK, NSL, D+1) -> (NSL, 128, CK, D+1) with sk = c*128 + p
        vb = np.ascontiguousarray(
            vext.reshape(CK, 128, NSL, D + 1).transpose(2, 1, 0, 3)
        ).astype(bf16)
        in_maps.append({"qb": qb, "kb": kb, "vb": vb})
    return in_maps


def _gather(results):
    out = np.empty((B, SQ, H, D), dtype=np.float32)
    for core in range(NCORES):
        b = core // (NCORES // B)
        h0 = (core % (NCORES // B)) * NSL
        out[b, :, h0 : h0 + NSL, :] = results[core]["out"].transpose(1, 0, 2)
    return out


def kernel(q, kv, attn_bias, key_padding_mask):
    from concourse.bass_utils import run_bass_kernel_spmd

    nc = _get_nc()
    in_maps = _make_in_maps(q, kv, attn_bias, key_padding_mask)
    res = run_bass_kernel_spmd(nc, in_maps, list(range(NCORES)))
    return _gather(res.results)
